# revision 20
# baseline (speedup 1.0000x reference)
"""Trainium2 Bass kernel for nn_DirectedClassifier (sparse attention).

Sharding: batch across 8 cores (2 batches/core); weights replicated.

Per core, per batch:
  - LayerNorm token-major (stats via DVE reduce + ACT Square/Ln/Exp for
    rstd), one fused (x-mu)*rstd tensor_scalar, PE-transpose -> xhatT.
  - Projections as f32r matmuls from xhatT; LN gains and 1/sqrt(DK) are
    folded into the weights on the host, LN biases become K=1 matmul rows.
  - Scores q-major: PSUM = Qh^T K + A, where A = -500*(mask+I) (bf16,
    host-precomputed) streams in through an identity-weight matmul.
    ACT Exp reads PSUM directly with fused row-sum (accum_out); masked
    entries underflow to exactly 0.
  - attn = p * recip(rowsum) (DVE), DMA'd out fp32; a bf16 copy (GPSIMD)
    is PE-transposed for the PV matmul.
  - PV per head: ctxT = V_h^T @ pT (bf16, fp32 accumulate), FC from ctxT
    (f32r).  The reference's 1e-13 renorm epsilon is ~2e-13 relative and
    is ignored.
"""

import numpy as np

B, L, D, H, DK, DV, DM = 16, 1024, 512, 8, 64, 64, 512
N_CORES = 8
B_LOC = B // N_CORES
LN_EPS = 1e-5
MASK_NEG = -240.0  # max-magnitude finite in ml float8_e4m3; exp(s-240)==0.0 in fp32
NT = L // 128      # 8 token tiles
ND = D // 128      # 4 channel blocks

_CACHE = {}


def _split_multi_waits(nc):
    """The walrus build in this container rejects instructions carrying
    more than one sync-wait; hoist extras onto standalone EventSemaphore
    instructions (same engine, immediately before — same ordering)."""
    import concourse.mybir as mybir
    for fn in nc.m.functions:
        for blk in fn.blocks:
            out = []
            for inst in blk.instructions:
                si = inst.sync_info
                if si is not None and len(si.on_wait) > 1:
                    for j, w in enumerate(si.on_wait):
                        out.append(mybir.InstEventSemaphore(
                            name=f"{inst.name}-ws{j}", engine=inst.engine,
                            ins=[], outs=[],
                            sync_info=mybir.SyncInfo(on_wait=[w],
                                                     on_update=[])))
                    inst.sync_info = mybir.SyncInfo(
                        on_wait=[], on_update=list(si.on_update))
                out.append(inst)
            blk.instructions = out


def _build_nc(repeat=1):
    import contextlib
    import concourse.bass as bass
    import concourse.mybir as mybir
    from concourse.tile import TileContext

    F32 = mybir.dt.float32
    F32R = mybir.dt.float32r
    BF16 = mybir.dt.bfloat16
    FP8 = mybir.dt.float8e4
    AF = mybir.ActivationFunctionType
    ALU = mybir.AluOpType
    AX = mybir.AxisListType

    nc = bass.Bass(target_bir_lowering=False, trn_type="TRN2")

    xq = nc.dram_tensor("xq", [B_LOC, L, D], F32, kind="ExternalInput")
    xk = nc.dram_tensor("xk", [B_LOC, L, D], F32, kind="ExternalInput")
    xv = nc.dram_tensor("xv", [B_LOC, L, D], F32, kind="ExternalInput")
    amask = nc.dram_tensor("amask", [B_LOC, L, L], FP8, kind="ExternalInput")
    wqT = nc.dram_tensor("wqT", [D, H * DK], F32, kind="ExternalInput")
    wkT = nc.dram_tensor("wkT", [D, H * DK], F32, kind="ExternalInput")
    wvT = nc.dram_tensor("wvT", [D, H * DV], F32, kind="ExternalInput")
    wfcT = nc.dram_tensor("wfcT", [H * DV, DM], F32, kind="ExternalInput")
    # bias rows [1, 3*512]: (Wq@b1/sqrt(DK) | Wk@b2 | Wv@b3)
    bqkv = nc.dram_tensor("bqkv", [1, 3 * D], F32, kind="ExternalInput")
    eye_f = nc.dram_tensor("eye_f", [128, 128], F32, kind="ExternalInput")
    eye_b = nc.dram_tensor("eye_b", [128, 128], BF16, kind="ExternalInput")
    eye_8 = nc.dram_tensor("eye_8", [128, 128], FP8, kind="ExternalInput")

    dyn = nc.dram_tensor("dyn", [B_LOC, L, DM], F32, kind="ExternalOutput")
    attn = nc.dram_tensor("attn", [B_LOC, H, L, L], F32, kind="ExternalOutput")

    with TileContext(nc) as tc:
        with contextlib.ExitStack() as ctx:
            ep = ctx.enter_context
            wpool = ep(tc.tile_pool(name="weights", bufs=1))
            ppool = ep(tc.tile_pool(name="persist", bufs=1))
            pppool = ep(tc.tile_pool(name="ppbuf", bufs=3))
            xin = ep(tc.tile_pool(name="xin", bufs=1))
            work = ep(tc.tile_pool(name="work", bufs=2))
            stat = ep(tc.tile_pool(name="stat", bufs=2))
            outp = ep(tc.tile_pool(name="outp", bufs=2))
            ps_big = ep(tc.tile_pool(name="ps_big", bufs=2, space="PSUM"))
            ps_pv = ep(tc.tile_pool(name="ps_pv", bufs=1, space="PSUM"))
            ps_sm = ep(tc.tile_pool(name="ps_sm", bufs=2, space="PSUM"))

            # ---- constants / weights (f32r via DVE rounding copies) ----
            eyef_t = wpool.tile([128, 128], F32, tag="eyef")
            nc.sync.dma_start(eyef_t[:], eye_f[:])
            eyeb_t = wpool.tile([128, 128], BF16, tag="eyeb")
            nc.sync.dma_start(eyeb_t[:], eye_b[:])
            eye8_t = wpool.tile([128, 128], FP8, tag="eye8")
            nc.sync.dma_start(eye8_t[:], eye_8[:])

            w_r = {}

            def load_weight(name, dram):
                wr = wpool.tile([128, ND * 512], F32R, tag=name)
                for n in range(ND):
                    raw = outp.tile([128, 512], F32, tag="attn_f32")
                    nc.sync.dma_start(raw[:],
                                      dram[n * 128:(n + 1) * 128, :])
                    nc.vector.tensor_copy(wr[:, n * 512:(n + 1) * 512],
                                          raw[:])
                w_r[name] = wr

            deferred_w = [("wk", wkT), ("wv", wvT), ("wfc", wfcT)]
            load_weight("wq", wqT)

            b_r = wpool.tile([1, 3 * D], F32R, tag="bias")
            for n in range(3):
                braw = xin.tile([1, D], F32, tag="xload")
                nc.sync.dma_start(braw[:], bqkv[:, n * D:(n + 1) * D])
                nc.vector.tensor_copy(b_r[:, n * D:(n + 1) * D], braw[:])
            onesf = xin.tile([1, 512], F32, tag="xload")
            nc.vector.memset(onesf[:], 1.0)
            ones_r = wpool.tile([1, 512], F32R, tag="ones")
            nc.vector.tensor_copy(ones_r[:], onesf[:])
            eps_t = wpool.tile([128, 1], F32, tag="eps")
            nc.vector.memset(eps_t[:], LN_EPS)

            for b_iter in range(B_LOC * repeat):
                b = b_iter % B_LOC
                # ========== LN + transpose + projection, per tensor ==========
                qT = ppool.tile([128, ND * L], F32R, tag="qT")
                kT = ppool.tile([128, ND * L], F32R, tag="kT")
                vN = ppool.tile([128, NT * 512], BF16, tag="vN")

                for tname, dram in (("q", xq), ("k", xk), ("v", xv)):
                    # xhT block di at cols [di*L, (di+1)*L)
                    xhT = ppool.tile([128, ND * L], F32R, tag="xhT")
                    xfull = xin.tile([128, NT * D], F32, tag="xfull")
                    mean = stat.tile([128, NT], F32, tag="mean")
                    ssqs = stat.tile([128, NT], F32, tag="ssq")
                    for tj in range(NT):
                        xt = xfull[:, tj * D:(tj + 1) * D]
                        nc.sync.dma_start(xt,
                                          dram[b, tj * 128:(tj + 1) * 128, :])
                        ssum = stat.tile([128, 1], F32, tag="ssum")
                        nc.vector.tensor_reduce(ssum[:], xt,
                                                axis=AX.X, op=ALU.add)
                        nc.vector.tensor_scalar_mul(mean[:, tj:tj + 1],
                                                    ssum[:], 1.0 / D)
                        sq = work.tile([128, D], F32, tag="lnwork")
                        nc.scalar.activation(sq[:], xt, AF.Square,
                                             accum_out=ssqs[:, tj:tj + 1])
                    if deferred_w:  # overlap weight loads with first LN
                        for nm, dr in deferred_w:
                            load_weight(nm, dr)
                        deferred_w.clear()
                    msq = stat.tile([128, NT], F32, tag="msq")
                    nc.vector.tensor_mul(msq[:], mean[:], mean[:])
                    var = stat.tile([128, NT], F32, tag="var")
                    nc.vector.tensor_scalar_mul(var[:], ssqs[:], 1.0 / D)
                    nc.vector.tensor_sub(var[:], var[:], msq[:])
                    lnt = stat.tile([128, NT], F32, tag="lnt")
                    nc.scalar.activation(lnt[:], var[:], AF.Ln,
                                         bias=eps_t[:])
                    rstd = stat.tile([128, NT], F32, tag="rstd")
                    nc.scalar.activation(rstd[:], lnt[:], AF.Exp,
                                         scale=-0.5)
                    for tj in range(NT):
                        xhat = work.tile([128, D], F32, tag="lnwork")
                        nc.vector.tensor_scalar(xhat[:],
                                                xfull[:, tj * D:(tj + 1) * D],
                                                mean[:, tj:tj + 1],
                                                rstd[:, tj:tj + 1],
                                                ALU.subtract, ALU.mult)
                        pt = ps_sm.tile([128, 512], F32, tag="ps")
                        for di in range(ND):
                            nc.tensor.transpose(
                                pt[:, di * 128:(di + 1) * 128],
                                xhat[:, di * 128:(di + 1) * 128], eyef_t[:])
                        # evac as one [128,512] copy into a column-strided AP
                        dst = xhT[:].rearrange("p (di c) -> p di c", di=ND)
                        nc.scalar.copy(
                            dst[:, :, tj * 128:(tj + 1) * 128],
                            pt[:].rearrange("p (di c) -> p di c", di=ND))

                    if tname in ("q", "k"):
                        wr = w_r["wq"] if tname == "q" else w_r["wk"]
                        boff = 0 if tname == "q" else D
                        dstT = qT if tname == "q" else kT
                        for ci in range(ND):
                            for qc in range(2):
                                pp = ps_sm.tile([128, 512], F32, tag="ps")
                                for di in range(ND):
                                    nc.tensor.matmul(
                                        pp[:],
                                        wr[:, di * 512 + ci * 128:
                                           di * 512 + (ci + 1) * 128],
                                        xhT[:, di * L + qc * 512:
                                            di * L + (qc + 1) * 512],
                                        start=(di == 0), stop=False)
                                nc.tensor.matmul(
                                    pp[:],
                                    b_r[:, boff + ci * 128:
                                        boff + (ci + 1) * 128],
                                    ones_r[:, 0:512],
                                    start=False, stop=True)
                                nc.scalar.copy(
                                    dstT[:, ci * L + qc * 512:
                                         ci * L + (qc + 1) * 512], pp[:])
                    else:  # v -> natural layout, bf16
                        for tj in range(NT):
                            pp = ps_sm.tile([128, 512], F32, tag="ps")
                            for di in range(ND):
                                nc.tensor.matmul(
                                    pp[:],
                                    xhT[:, di * L + tj * 128:
                                        di * L + (tj + 1) * 128],
                                    w_r["wv"][:, di * 512:(di + 1) * 512],
                                    start=(di == 0), stop=False)
                            nc.tensor.matmul(
                                pp[:], ones_r[:, 0:128],
                                b_r[:, 2 * D:2 * D + 512],
                                start=False, stop=True)
                            nc.scalar.copy(vN[:, tj * 512:(tj + 1) * 512],
                                           pp[:])

                # ========== A mask (bf16, host-precomputed) ==========
                a_t = ppool.tile([128, NT * L], FP8, tag="amask")
                for n in range(NT):
                    nc.sync.dma_start(a_t[:, n * L:(n + 1) * L],
                                      amask[b, n * 128:(n + 1) * 128, :])

                # ========== attention ==========
                ctxT = ppool.tile([128, ND * L], F32R, tag="ctxT")
                for ci in range(ND):  # head pair (2*ci, 2*ci+1)
                    pbfs, rrs = [], []
                    for hh in range(2):
                        h = 2 * ci + hh
                        half = hh * 64
                        p_bf = pppool.tile([128, NT * L], BF16, tag="pp")
                        pbfs.append(p_bf)
                        rs = stat.tile([128, NT], F32, tag="rowsum")
                        rr = stat.tile([128, NT], F32, tag="rowrec")
                        rrs.append(rr)
                        for qb in range(NT):
                            psc = ps_big.tile([128, 1024], F32, tag="scores")
                            for kc in range(2):
                                reg = psc[:, kc * 512:(kc + 1) * 512]
                                nc.tensor.matmul(
                                    reg,
                                    qT[half:half + 64,
                                       ci * L + qb * 128:
                                       ci * L + (qb + 1) * 128],
                                    kT[half:half + 64,
                                       ci * L + kc * 512:
                                       ci * L + (kc + 1) * 512],
                                    start=True, stop=False,
                                    tile_position=(half, 0))
                                nc.tensor.matmul(
                                    reg, eye8_t[:],
                                    a_t[:, qb * L + kc * 512:
                                        qb * L + (kc + 1) * 512],
                                    start=False, stop=True)
                            a_f = outp.tile([128, 1024], F32, tag="attn_f32")
                            nc.scalar.activation(a_f[:], psc[:], AF.Exp,
                                                 accum_out=rs[:, qb:qb + 1])
                            nc.vector.reciprocal(rr[:, qb:qb + 1],
                                                 rs[:, qb:qb + 1])
                            nc.vector.tensor_scalar_mul(a_f[:], a_f[:],
                                                        rr[:, qb:qb + 1])
                            nc.sync.dma_start(
                                attn[b, h, qb * 128:(qb + 1) * 128, :],
                                a_f[:])
                            nc.gpsimd.tensor_copy(
                                p_bf[:, qb * L:(qb + 1) * L], a_f[:])
                    # transpose both heads' p (bf16), kb-major
                    pTs = []
                    for hh in range(2):
                        pT = pppool.tile([128, NT * L], BF16, tag="pp")
                        pTs.append(pT)
                        for kb in range(NT):
                            ptp = ps_sm.tile([128, 1024], BF16, tag="ps")
                            for qb in range(NT):
                                nc.tensor.transpose(
                                    ptp[:, qb * 128:(qb + 1) * 128],
                                    pbfs[hh][:, qb * L + kb * 128:
                                             qb * L + (kb + 1) * 128],
                                    eyeb_t[:])
                            nc.vector.tensor_copy(
                                pT[:, kb * L:(kb + 1) * L], ptp[:])
                    # PV both heads, col-packed; separate banks per head
                    for qc in range(2):
                        ppv = ps_pv.tile([128, 1024], F32, tag="pv")
                        for kb in range(NT):
                            for hh in range(2):
                                h = 2 * ci + hh
                                nc.tensor.matmul(
                                    ppv[hh * 64:hh * 64 + 64,
                                        hh * 512:hh * 512 + 512],
                                    vN[:, kb * 512 + h * 64:
                                       kb * 512 + h * 64 + 64],
                                    pTs[hh][:, kb * L + qc * 512:
                                            kb * L + (qc + 1) * 512],
                                    start=(kb == 0), stop=(kb == NT - 1),
                                    tile_position=(0, hh * 64))
                        for hh in range(2):
                            nc.scalar.copy(
                                ctxT[hh * 64:hh * 64 + 64,
                                     ci * L + qc * 512:ci * L + (qc + 1) * 512],
                                ppv[hh * 64:hh * 64 + 64,
                                    hh * 512:hh * 512 + 512])

                # ========== FC ==========
                for tj in range(NT):
                    pf = ps_sm.tile([128, 512], F32, tag="ps")
                    for ci2 in range(ND):
                        nc.tensor.matmul(
                            pf[:],
                            ctxT[:, ci2 * L + tj * 128:
                                 ci2 * L + (tj + 1) * 128],
                            w_r["wfc"][:, ci2 * 512:(ci2 + 1) * 512],
                            start=(ci2 == 0), stop=(ci2 == ND - 1))
                    df = outp.tile([128, 512], F32, tag="dyn_f32")
                    nc.scalar.copy(df[:], pf[:])
                    nc.sync.dma_start(dyn[b, tj * 128:(tj + 1) * 128, :],
                                      df[:])

    return nc


def _host_prep(Wq, Wk, Wv, Wfc, g1, b1, g2, b2, g3, b3):
    import ml_dtypes
    s = 1.0 / np.sqrt(np.float32(DK))
    WqE = (Wq * g1[None, :] * s).astype(np.float32)
    WkE = (Wk * g2[None, :]).astype(np.float32)
    WvE = (Wv * g3[None, :]).astype(np.float32)
    bias = np.concatenate([(Wq @ b1) * s, Wk @ b2, Wv @ b3])
    return {
        "wqT": np.ascontiguousarray(WqE.T),
        "wkT": np.ascontiguousarray(WkE.T),
        "wvT": np.ascontiguousarray(WvE.T),
        "wfcT": np.ascontiguousarray(Wfc.T.astype(np.float32)),
        "bqkv": bias.astype(np.float32)[None, :],
        "eye_f": np.eye(128, dtype=np.float32),
        "eye_b": np.eye(128, dtype=ml_dtypes.bfloat16),
        "eye_8": np.eye(128, dtype=ml_dtypes.float8_e4m3),
    }


def _make_amask(mask):
    """A = MASK_NEG*min(mask + I, 1), fp8 e4m3 (0 and -240 are exact)."""
    import ml_dtypes
    a = np.minimum(mask.astype(np.float32)
                   + np.eye(L, dtype=np.float32)[None], 1.0)
    return (a * MASK_NEG).astype(ml_dtypes.float8_e4m3)


def get_nc(split=True, repeat=1):
    key = ("nc", split, repeat)
    if key not in _CACHE:
        nc = _build_nc(repeat=repeat)
        if split:
            _split_multi_waits(nc)
        _CACHE[key] = nc
    return _CACHE[key]


def make_in_maps(q, k, v, mask, Wq, Wk, Wv, Wfc,
                 g1, b1, g2, b2, g3, b3):
    shared = _host_prep(
        np.asarray(Wq, np.float32), np.asarray(Wk, np.float32),
        np.asarray(Wv, np.float32), np.asarray(Wfc, np.float32),
        np.asarray(g1, np.float32), np.asarray(b1, np.float32),
        np.asarray(g2, np.float32), np.asarray(b2, np.float32),
        np.asarray(g3, np.float32), np.asarray(b3, np.float32))
    amask_all = _make_amask(np.asarray(mask))
    q = np.asarray(q, np.float32)
    k = np.asarray(k, np.float32)
    v = np.asarray(v, np.float32)
    in_maps = []
    for c in range(N_CORES):
        sl = slice(c * B_LOC, (c + 1) * B_LOC)
        m = dict(shared)
        m["xq"] = q[sl]
        m["xk"] = k[sl]
        m["xv"] = v[sl]
        m["amask"] = amask_all[sl]
        in_maps.append(m)
    return in_maps


def assemble(results):
    dynamic = np.empty((B, L, DM), np.float32)
    attn_out = np.empty((H * B, L, L), np.float32)
    for c in range(N_CORES):
        r = results[c]
        dynamic[c * B_LOC:(c + 1) * B_LOC] = r["dyn"]
        for i in range(B_LOC):
            attn_out[c * B_LOC + i::B] = r["attn"][i]
    return dynamic, attn_out


def kernel(q, k, v, mask, Wq, Wk, Wv, Wfc, g1, b1, g2, b2, g3, b3):
    from concourse import bass_utils
    nc = get_nc()
    in_maps = make_in_maps(q, k, v, mask, Wq, Wk, Wv, Wfc,
                           g1, b1, g2, b2, g3, b3)
    res = bass_utils.run_bass_kernel_spmd(nc, in_maps,
                                          core_ids=list(range(N_CORES)))
    return assemble(res.results)


# revision 22
# speedup vs baseline: 1.3417x; 1.3417x over previous
"""Trainium2 Bass kernel for nn_DirectedClassifier (sparse attention).

Sharding: batch across 8 cores (2 batches/core); weights replicated.

Per core, per batch:
  - LayerNorm token-major (stats via DVE reduce + ACT Square/Ln/Exp for
    rstd), one fused (x-mu)*rstd tensor_scalar, PE-transpose -> xhatT.
  - Projections as f32r matmuls from xhatT; LN gains and 1/sqrt(DK) are
    folded into the weights on the host, LN biases become K=1 matmul rows.
  - Scores q-major: PSUM = Qh^T K + A, where A = -500*(mask+I) (bf16,
    host-precomputed) streams in through an identity-weight matmul.
    ACT Exp reads PSUM directly with fused row-sum (accum_out); masked
    entries underflow to exactly 0.
  - attn = p * recip(rowsum) (DVE), DMA'd out fp32; a bf16 copy (GPSIMD)
    is PE-transposed for the PV matmul.
  - PV per head: ctxT = V_h^T @ pT (bf16, fp32 accumulate), FC from ctxT
    (f32r).  The reference's 1e-13 renorm epsilon is ~2e-13 relative and
    is ignored.
"""

import numpy as np

B, L, D, H, DK, DV, DM = 16, 1024, 512, 8, 64, 64, 512
N_CORES = 8
B_LOC = B // N_CORES
LN_EPS = 1e-5
MASK_NEG = -240.0  # max-magnitude finite in ml float8_e4m3; exp(s-240)==0.0 in fp32
NT = L // 128      # 8 token tiles
ND = D // 128      # 4 channel blocks

_CACHE = {}


def _split_multi_waits(nc):
    """The walrus build in this container rejects instructions carrying
    more than one sync-wait; hoist extras onto standalone EventSemaphore
    instructions (same engine, immediately before — same ordering)."""
    import concourse.mybir as mybir
    for fn in nc.m.functions:
        for blk in fn.blocks:
            out = []
            for inst in blk.instructions:
                si = inst.sync_info
                if si is not None and len(si.on_wait) > 1:
                    for j, w in enumerate(si.on_wait):
                        out.append(mybir.InstEventSemaphore(
                            name=f"{inst.name}-ws{j}", engine=inst.engine,
                            ins=[], outs=[],
                            sync_info=mybir.SyncInfo(on_wait=[w],
                                                     on_update=[])))
                    inst.sync_info = mybir.SyncInfo(
                        on_wait=[], on_update=list(si.on_update))
                out.append(inst)
            blk.instructions = out


def _build_nc(repeat=1):
    import contextlib
    import concourse.bass as bass
    import concourse.mybir as mybir
    from concourse.tile import TileContext

    F32 = mybir.dt.float32
    F32R = mybir.dt.float32r
    BF16 = mybir.dt.bfloat16
    FP8 = mybir.dt.float8e4
    AF = mybir.ActivationFunctionType
    ALU = mybir.AluOpType
    AX = mybir.AxisListType

    nc = bass.Bass(target_bir_lowering=False, trn_type="TRN2")

    xq = nc.dram_tensor("xq", [B_LOC, L, D], F32, kind="ExternalInput")
    xk = nc.dram_tensor("xk", [B_LOC, L, D], F32, kind="ExternalInput")
    xv = nc.dram_tensor("xv", [B_LOC, L, D], F32, kind="ExternalInput")
    amask = nc.dram_tensor("amask", [B_LOC, L, L], FP8, kind="ExternalInput")
    wqT = nc.dram_tensor("wqT", [D, H * DK], F32, kind="ExternalInput")
    wkT = nc.dram_tensor("wkT", [D, H * DK], F32, kind="ExternalInput")
    wvT = nc.dram_tensor("wvT", [D, H * DV], F32, kind="ExternalInput")
    wfcT = nc.dram_tensor("wfcT", [H * DV, DM], F32, kind="ExternalInput")
    # bias rows [1, 3*512]: (Wq@b1/sqrt(DK) | Wk@b2 | Wv@b3)
    bqkv = nc.dram_tensor("bqkv", [1, 3 * D], F32, kind="ExternalInput")
    eye_f = nc.dram_tensor("eye_f", [128, 128], F32, kind="ExternalInput")
    eye_b = nc.dram_tensor("eye_b", [128, 128], BF16, kind="ExternalInput")
    eye_8 = nc.dram_tensor("eye_8", [128, 128], FP8, kind="ExternalInput")

    dyn = nc.dram_tensor("dyn", [B_LOC, L, DM], F32, kind="ExternalOutput")
    attn = nc.dram_tensor("attn", [B_LOC, H, L, L], F32, kind="ExternalOutput")

    with TileContext(nc) as tc:
        with contextlib.ExitStack() as ctx:
            ep = ctx.enter_context
            wpool = ep(tc.tile_pool(name="weights", bufs=1))
            ppool = ep(tc.tile_pool(name="persist", bufs=1))
            pppool = ep(tc.tile_pool(name="ppbuf", bufs=3))
            xin = ep(tc.tile_pool(name="xin", bufs=1))
            work = ep(tc.tile_pool(name="work", bufs=2))
            stat = ep(tc.tile_pool(name="stat", bufs=2))
            outp = ep(tc.tile_pool(name="outp", bufs=2))
            ps_big = ep(tc.tile_pool(name="ps_big", bufs=2, space="PSUM"))
            ps_pv = ep(tc.tile_pool(name="ps_pv", bufs=1, space="PSUM"))
            ps_sm = ep(tc.tile_pool(name="ps_sm", bufs=2, space="PSUM"))

            # ---- constants / weights (f32r via DVE rounding copies) ----
            eyef_t = wpool.tile([128, 128], F32, tag="eyef")
            nc.sync.dma_start(eyef_t[:], eye_f[:])
            eyeb_t = wpool.tile([128, 128], BF16, tag="eyeb")
            nc.sync.dma_start(eyeb_t[:], eye_b[:])
            eye8_t = wpool.tile([128, 128], FP8, tag="eye8")
            nc.sync.dma_start(eye8_t[:], eye_8[:])

            w_r = {}

            def load_weight(name, dram):
                wr = wpool.tile([128, ND * 512], F32R, tag=name)
                for n in range(ND):
                    raw = outp.tile([128, 512], F32, tag="attn_f32")
                    nc.sync.dma_start(raw[:],
                                      dram[n * 128:(n + 1) * 128, :])
                    nc.vector.tensor_copy(wr[:, n * 512:(n + 1) * 512],
                                          raw[:])
                w_r[name] = wr

            deferred_w = [("wk", wkT), ("wv", wvT), ("wfc", wfcT)]
            load_weight("wq", wqT)

            b_r = wpool.tile([1, 3 * D], F32R, tag="bias")
            for n in range(3):
                braw = xin.tile([1, D], F32, tag="xload")
                nc.sync.dma_start(braw[:], bqkv[:, n * D:(n + 1) * D])
                nc.vector.tensor_copy(b_r[:, n * D:(n + 1) * D], braw[:])
            onesf = xin.tile([1, 512], F32, tag="xload")
            nc.vector.memset(onesf[:], 1.0)
            ones_r = wpool.tile([1, 512], F32R, tag="ones")
            nc.vector.tensor_copy(ones_r[:], onesf[:])
            eps_t = wpool.tile([128, 1], F32, tag="eps")
            nc.vector.memset(eps_t[:], LN_EPS)

            for b_iter in range(B_LOC * repeat):
                b = b_iter % B_LOC
                # ========== LN + transpose + projection, per tensor ==========
                qT = ppool.tile([128, ND * L], F32R, tag="qT")
                kT = ppool.tile([128, ND * L], F32R, tag="kT")
                vN = ppool.tile([128, NT * 512], BF16, tag="vN")

                for tname, dram in (("q", xq), ("k", xk), ("v", xv)):
                    # xhT block di at cols [di*L, (di+1)*L)
                    xhT = ppool.tile([128, ND * L], F32R, tag="xhT")
                    xfull = xin.tile([128, NT * D], F32, tag="xfull")
                    mean = stat.tile([128, NT], F32, tag="mean")
                    ssqs = stat.tile([128, NT], F32, tag="ssq")
                    for tj in range(NT):
                        xt = xfull[:, tj * D:(tj + 1) * D]
                        nc.sync.dma_start(xt,
                                          dram[b, tj * 128:(tj + 1) * 128, :])
                        ssum = stat.tile([128, 1], F32, tag="ssum")
                        nc.vector.tensor_reduce(ssum[:], xt,
                                                axis=AX.X, op=ALU.add)
                        nc.vector.tensor_scalar_mul(mean[:, tj:tj + 1],
                                                    ssum[:], 1.0 / D)
                        sq = work.tile([128, D], F32, tag="lnwork")
                        nc.scalar.activation(sq[:], xt, AF.Square,
                                             accum_out=ssqs[:, tj:tj + 1])
                    if deferred_w:  # overlap weight loads with first LN
                        for nm, dr in deferred_w:
                            load_weight(nm, dr)
                        deferred_w.clear()
                    msq = stat.tile([128, NT], F32, tag="msq")
                    nc.vector.tensor_mul(msq[:], mean[:], mean[:])
                    var = stat.tile([128, NT], F32, tag="var")
                    nc.vector.tensor_scalar_mul(var[:], ssqs[:], 1.0 / D)
                    nc.vector.tensor_sub(var[:], var[:], msq[:])
                    lnt = stat.tile([128, NT], F32, tag="lnt")
                    nc.scalar.activation(lnt[:], var[:], AF.Ln,
                                         bias=eps_t[:])
                    rstd = stat.tile([128, NT], F32, tag="rstd")
                    nc.scalar.activation(rstd[:], lnt[:], AF.Exp,
                                         scale=-0.5)
                    for tj in range(NT):
                        xhat = work.tile([128, D], F32, tag="lnwork")
                        nc.vector.tensor_scalar(xhat[:],
                                                xfull[:, tj * D:(tj + 1) * D],
                                                mean[:, tj:tj + 1],
                                                rstd[:, tj:tj + 1],
                                                ALU.subtract, ALU.mult)
                        pt = ps_sm.tile([128, 512], F32, tag="ps")
                        for di in range(ND):
                            nc.tensor.transpose(
                                pt[:, di * 128:(di + 1) * 128],
                                xhat[:, di * 128:(di + 1) * 128], eyef_t[:])
                        # evac as one [128,512] copy into a column-strided AP
                        dst = xhT[:].rearrange("p (di c) -> p di c", di=ND)
                        nc.scalar.copy(
                            dst[:, :, tj * 128:(tj + 1) * 128],
                            pt[:].rearrange("p (di c) -> p di c", di=ND))

                    if tname in ("q", "k"):
                        wr = w_r["wq"] if tname == "q" else w_r["wk"]
                        boff = 0 if tname == "q" else D
                        dstT = qT if tname == "q" else kT
                        for ci in range(ND):
                            for qc in range(2):
                                pp = ps_sm.tile([128, 512], F32, tag="ps")
                                for di in range(ND):
                                    nc.tensor.matmul(
                                        pp[:],
                                        wr[:, di * 512 + ci * 128:
                                           di * 512 + (ci + 1) * 128],
                                        xhT[:, di * L + qc * 512:
                                            di * L + (qc + 1) * 512],
                                        start=(di == 0), stop=False)
                                nc.tensor.matmul(
                                    pp[:],
                                    b_r[:, boff + ci * 128:
                                        boff + (ci + 1) * 128],
                                    ones_r[:, 0:512],
                                    start=False, stop=True)
                                nc.scalar.copy(
                                    dstT[:, ci * L + qc * 512:
                                         ci * L + (qc + 1) * 512], pp[:])
                    else:  # v -> natural layout, bf16
                        for tj in range(NT):
                            pp = ps_sm.tile([128, 512], F32, tag="ps")
                            for di in range(ND):
                                nc.tensor.matmul(
                                    pp[:],
                                    xhT[:, di * L + tj * 128:
                                        di * L + (tj + 1) * 128],
                                    w_r["wv"][:, di * 512:(di + 1) * 512],
                                    start=(di == 0), stop=False)
                            nc.tensor.matmul(
                                pp[:], ones_r[:, 0:128],
                                b_r[:, 2 * D:2 * D + 512],
                                start=False, stop=True)
                            nc.scalar.copy(vN[:, tj * 512:(tj + 1) * 512],
                                           pp[:])

                # ========== A mask (bf16, host-precomputed) ==========
                a_t = ppool.tile([128, NT * L], FP8, tag="amask")
                for n in range(NT):
                    nc.scalar.dma_start(a_t[:, n * L:(n + 1) * L],
                                        amask[b, n * 128:(n + 1) * 128, :])

                # ========== attention ==========
                ctxT = ppool.tile([128, ND * L], F32R, tag="ctxT")
                for ci in range(ND):  # head pair (2*ci, 2*ci+1)
                    pbfs, rrs = [], []
                    for hh in range(2):
                        h = 2 * ci + hh
                        half = hh * 64
                        p_bf = pppool.tile([128, NT * L], BF16, tag="pp")
                        pbfs.append(p_bf)
                        rs = stat.tile([128, NT], F32, tag="rowsum")
                        rr = stat.tile([128, NT], F32, tag="rowrec")
                        rrs.append(rr)
                        for qb in range(NT):
                            psc = ps_big.tile([128, 1024], F32, tag="scores")
                            # group same-lhsT matmuls (QK kc-pair, then A-adds)
                            for kc in range(2):
                                nc.tensor.matmul(
                                    psc[:, kc * 512:(kc + 1) * 512],
                                    qT[half:half + 64,
                                       ci * L + qb * 128:
                                       ci * L + (qb + 1) * 128],
                                    kT[half:half + 64,
                                       ci * L + kc * 512:
                                       ci * L + (kc + 1) * 512],
                                    start=True, stop=False,
                                    tile_position=(half, 0))
                            for kc in range(2):
                                nc.tensor.matmul(
                                    psc[:, kc * 512:(kc + 1) * 512],
                                    eye8_t[:],
                                    a_t[:, qb * L + kc * 512:
                                        qb * L + (kc + 1) * 512],
                                    start=False, stop=True)
                            a_f = outp.tile([128, 1024], F32, tag="attn_f32")
                            nc.scalar.activation(a_f[:], psc[:], AF.Exp,
                                                 accum_out=rs[:, qb:qb + 1])
                            nc.vector.reciprocal(rr[:, qb:qb + 1],
                                                 rs[:, qb:qb + 1])
                            nc.vector.tensor_scalar_mul(a_f[:], a_f[:],
                                                        rr[:, qb:qb + 1])
                            dma_eng = nc.sync if qb % 2 == 0 else nc.scalar
                            dma_eng.dma_start(
                                attn[b, h, qb * 128:(qb + 1) * 128, :],
                                a_f[:])
                            nc.gpsimd.tensor_copy(
                                p_bf[:, qb * L:(qb + 1) * L], a_f[:])
                    # transpose both heads' p (bf16), kb-major
                    pTs = []
                    for hh in range(2):
                        pT = pppool.tile([128, NT * L], BF16, tag="pp")
                        pTs.append(pT)
                        for kb in range(NT):
                            ptp = ps_sm.tile([128, 1024], BF16, tag="ps")
                            for qb in range(NT):
                                nc.tensor.transpose(
                                    ptp[:, qb * 128:(qb + 1) * 128],
                                    pbfs[hh][:, qb * L + kb * 128:
                                             qb * L + (kb + 1) * 128],
                                    eyeb_t[:])
                            nc.vector.tensor_copy(
                                pT[:, kb * L:(kb + 1) * L], ptp[:])
                    # PV both heads, col-packed; separate banks per head
                    for qc in range(2):
                        ppv = ps_pv.tile([128, 1024], F32, tag="pv")
                        for kb in range(NT):
                            for hh in range(2):
                                h = 2 * ci + hh
                                nc.tensor.matmul(
                                    ppv[hh * 64:hh * 64 + 64,
                                        hh * 512:hh * 512 + 512],
                                    vN[:, kb * 512 + h * 64:
                                       kb * 512 + h * 64 + 64],
                                    pTs[hh][:, kb * L + qc * 512:
                                            kb * L + (qc + 1) * 512],
                                    start=(kb == 0), stop=(kb == NT - 1),
                                    tile_position=(0, hh * 64))
                        for hh in range(2):
                            nc.scalar.copy(
                                ctxT[hh * 64:hh * 64 + 64,
                                     ci * L + qc * 512:ci * L + (qc + 1) * 512],
                                ppv[hh * 64:hh * 64 + 64,
                                    hh * 512:hh * 512 + 512])

                # ========== FC ==========
                for tj in range(NT):
                    pf = ps_sm.tile([128, 512], F32, tag="ps")
                    for ci2 in range(ND):
                        nc.tensor.matmul(
                            pf[:],
                            ctxT[:, ci2 * L + tj * 128:
                                 ci2 * L + (tj + 1) * 128],
                            w_r["wfc"][:, ci2 * 512:(ci2 + 1) * 512],
                            start=(ci2 == 0), stop=(ci2 == ND - 1))
                    df = outp.tile([128, 512], F32, tag="dyn_f32")
                    nc.scalar.copy(df[:], pf[:])
                    nc.scalar.dma_start(dyn[b, tj * 128:(tj + 1) * 128, :],
                                        df[:])

    return nc


def _host_prep(Wq, Wk, Wv, Wfc, g1, b1, g2, b2, g3, b3):
    import ml_dtypes
    s = 1.0 / np.sqrt(np.float32(DK))
    WqE = (Wq * g1[None, :] * s).astype(np.float32)
    WkE = (Wk * g2[None, :]).astype(np.float32)
    WvE = (Wv * g3[None, :]).astype(np.float32)
    bias = np.concatenate([(Wq @ b1) * s, Wk @ b2, Wv @ b3])
    return {
        "wqT": np.ascontiguousarray(WqE.T),
        "wkT": np.ascontiguousarray(WkE.T),
        "wvT": np.ascontiguousarray(WvE.T),
        "wfcT": np.ascontiguousarray(Wfc.T.astype(np.float32)),
        "bqkv": bias.astype(np.float32)[None, :],
        "eye_f": np.eye(128, dtype=np.float32),
        "eye_b": np.eye(128, dtype=ml_dtypes.bfloat16),
        "eye_8": np.eye(128, dtype=ml_dtypes.float8_e4m3),
    }


def _make_amask(mask):
    """A = MASK_NEG*min(mask + I, 1), fp8 e4m3 (0 and -240 are exact)."""
    import ml_dtypes
    a = np.minimum(mask.astype(np.float32)
                   + np.eye(L, dtype=np.float32)[None], 1.0)
    return (a * MASK_NEG).astype(ml_dtypes.float8_e4m3)


def get_nc(split=True, repeat=1):
    key = ("nc", split, repeat)
    if key not in _CACHE:
        nc = _build_nc(repeat=repeat)
        if split:
            _split_multi_waits(nc)
        _CACHE[key] = nc
    return _CACHE[key]


def make_in_maps(q, k, v, mask, Wq, Wk, Wv, Wfc,
                 g1, b1, g2, b2, g3, b3):
    shared = _host_prep(
        np.asarray(Wq, np.float32), np.asarray(Wk, np.float32),
        np.asarray(Wv, np.float32), np.asarray(Wfc, np.float32),
        np.asarray(g1, np.float32), np.asarray(b1, np.float32),
        np.asarray(g2, np.float32), np.asarray(b2, np.float32),
        np.asarray(g3, np.float32), np.asarray(b3, np.float32))
    amask_all = _make_amask(np.asarray(mask))
    q = np.asarray(q, np.float32)
    k = np.asarray(k, np.float32)
    v = np.asarray(v, np.float32)
    in_maps = []
    for c in range(N_CORES):
        sl = slice(c * B_LOC, (c + 1) * B_LOC)
        m = dict(shared)
        m["xq"] = q[sl]
        m["xk"] = k[sl]
        m["xv"] = v[sl]
        m["amask"] = amask_all[sl]
        in_maps.append(m)
    return in_maps


def assemble(results):
    dynamic = np.empty((B, L, DM), np.float32)
    attn_out = np.empty((H * B, L, L), np.float32)
    for c in range(N_CORES):
        r = results[c]
        dynamic[c * B_LOC:(c + 1) * B_LOC] = r["dyn"]
        for i in range(B_LOC):
            attn_out[c * B_LOC + i::B] = r["attn"][i]
    return dynamic, attn_out


def kernel(q, k, v, mask, Wq, Wk, Wv, Wfc, g1, b1, g2, b2, g3, b3):
    from concourse import bass_utils
    nc = get_nc()
    in_maps = make_in_maps(q, k, v, mask, Wq, Wk, Wv, Wfc,
                           g1, b1, g2, b2, g3, b3)
    res = bass_utils.run_bass_kernel_spmd(nc, in_maps,
                                          core_ids=list(range(N_CORES)))
    return assemble(res.results)
